# revision 3
# baseline (speedup 1.0000x reference)
"""Trainium2 Bass kernel v4: 5-tap Kaiser circular filter along H and W of a
(16, 3, 1024, 1024) fp32 tensor. Data-parallel over batch across 8 cores.

Per core: 2 batches x 3 channels = 6 independent (1024, 1024) slices.

v4 insight: HWDGE descriptor generation costs ~40ns/descriptor, serial per
ring -- the banded row layout (1 row per partition-chunk) needs ~2268
descriptors/slice and dominates everything. So the host materializes the
banded chunk layout in HBM:
  - x2[s, k, j, c] = x[s, (124j + k - 2) % 1024, (c - 2) % 1024]
    -> each partition k reads 9*1028 CONTIGUOUS elements: 128 descriptors
       of 37 KiB per slice, ONE load DMA (sync/SP ring).
  - y2[s, m, j, c] = y[s, 124j + m, c] (rows >= 1024 are wrapped dups)
    -> 124 contiguous 36 KiB descriptors, ONE store DMA (scalar/ACT ring);
       host transposes back and slices [:1024].
Compute: fused separable filter -- 5 accumulating float32r matmuls per
512-col PSUM block (lhsT = k[d]*A_H band matrix), PSUM evacuated by DVE.
"""

import numpy as np

B, C, H, W = 16, 3, 1024, 1024
N_CORES = 8
S = (B // N_CORES) * C  # slices per core
TAPS = 5
HALO = TAPS // 2  # 2
STRIDE = 124  # output rows per block
NBLK = 9  # ceil(1024 / 124)
CW = W + 2 * HALO  # chunk width 1028
PH = 2 + H + (128 - 2 - (H - 8 * STRIDE))  # 1120 padded rows (-2..1117)

_cache = {}


def _build_with_taps(kk, repeat=1, stages="full"):
    """kk: numpy [5] float32 tap weights. Returns compiled Bass object."""
    import concourse.bass as bass
    import concourse.bacc as bacc
    import concourse.mybir as mybir
    import concourse.tile as tile

    f32 = mybir.dt.float32
    f32r = mybir.dt.float32r
    nc = bacc.Bacc("TRN2", target_bir_lowering=False, debug=False, num_devices=N_CORES)

    x_d = nc.dram_tensor("x2", [S, 128, NBLK * CW], f32r, kind="ExternalInput")
    y_d = nc.dram_tensor("y2", [S, STRIDE, NBLK * W], f32, kind="ExternalOutput")
    a_d = nc.dram_tensor("afilt5", [128, TAPS * STRIDE], f32r, kind="ExternalInput")

    with tile.TileContext(nc) as tc:
        with (
            tc.tile_pool(name="wpool", bufs=1) as wpool,
            tc.tile_pool(name="inp", bufs=2) as inp,
            tc.tile_pool(name="psum", bufs=2, space="PSUM") as psum,
            tc.tile_pool(name="outp", bufs=2) as outp,
        ):
            a_s = wpool.tile([128, TAPS * STRIDE], f32r)
            nc.sync.dma_start(a_s[:], a_d[:])
            a3 = a_s.rearrange("p (d m) -> p d m", m=STRIDE)

            for _ in range(repeat):
                for s in range(S):
                    in_big = inp.tile([128, NBLK * CW], f32r)
                    in3 = in_big.rearrange("p (j c) -> p j c", c=CW)

                    # ONE load; per-partition 37 KiB split into two
                    # <=32 KiB contiguous descriptors (descriptor payload
                    # limit: a single 37 KiB descriptor wedges the core)
                    half_in = NBLK * CW // 2  # 4626
                    nc.sync.dma_start(
                        in_big[:, :],
                        bass.AP(
                            x_d,
                            s * 128 * NBLK * CW,
                            [[NBLK * CW, 128], [half_in, 2], [1, half_in]],
                        ),
                    )

                    out_big = outp.tile([STRIDE, NBLK * W], f32)
                    out3 = out_big.rearrange("p (j w) -> p j w", w=W)

                    if stages == "full":
                        # pairs of blocks share one 4-bank PSUM tile
                        for j0 in range(0, NBLK, 2):
                            npair = min(2, NBLK - j0)
                            ps = psum.tile([STRIDE, 2 * W], f32)
                            for b in range(npair):
                                for half in range(0, W, 512):
                                    for d in range(TAPS):
                                        nc.tensor.matmul(
                                            ps[:, b * W + half : b * W + half + 512],
                                            a3[:, d, :],
                                            in3[:, j0 + b, half + d : half + d + 512],
                                            start=(d == 0),
                                            stop=(d == TAPS - 1),
                                        )
                            ps3 = ps.rearrange("p (b w) -> p b w", w=W)
                            nc.vector.tensor_copy(
                                out3[:, j0 : j0 + npair, :], ps3[:, 0:npair, :]
                            )
                    else:  # "dma": loads + direct stores only
                        nc.vector.tensor_copy(
                            out3[:, :, :],
                            in3[0:STRIDE, :, HALO : HALO + W].bitcast(f32),
                        )

                    # ONE store; per-partition 36 KiB split into two
                    # <=32 KiB contiguous descriptors
                    half_out = NBLK * W // 2  # 4608
                    nc.scalar.dma_start(
                        bass.AP(
                            y_d,
                            s * STRIDE * NBLK * W,
                            [[NBLK * W, STRIDE], [half_out, 2], [1, half_out]],
                        ),
                        out_big[:, :],
                    )

    nc.compile()
    return nc


def _afilt_from_taps(kk):
    """[128, 5*124]: a5[k, d*124+m] = kk[d] * kk[dh] at k = m + 4 - dh."""
    a = np.zeros((128, TAPS * STRIDE), dtype=np.float32)
    for mcol in range(STRIDE):
        for dh in range(TAPS):
            k = mcol + 4 - dh
            if 0 <= k < 128:
                for d in range(TAPS):
                    a[k, d * STRIDE + mcol] = kk[d] * kk[dh]
    return a


def _pack_shard(shard):
    """[S, H, W] -> banded chunk layout [S, 128, NBLK*CW].

    x2[s, k, j*CW + c] = x[s, (124j + k - 2) % H, (c - 2) % W]
    """
    xp = np.pad(
        shard, ((0, 0), (HALO, PH - H - HALO), (HALO, CW - W - HALO)), mode="wrap"
    )  # [S, PH, CW]
    sb, rb, cb = xp.strides
    v = np.lib.stride_tricks.as_strided(
        xp, shape=(S, 128, NBLK, CW), strides=(sb, rb, STRIDE * rb, cb)
    )
    return np.ascontiguousarray(v).reshape(S, 128, NBLK * CW)


def _unpack_out(y2):
    """[S, STRIDE, NBLK*W] -> [S, H, W] (drop wrapped duplicate rows)."""
    return (
        y2.reshape(S, STRIDE, NBLK, W)
        .transpose(0, 2, 1, 3)
        .reshape(S, NBLK * STRIDE, W)[:, :H, :]
    )


def make_in_maps(x, kk):
    afilt = _afilt_from_taps(kk)
    per_core = B // N_CORES
    in_maps = []
    for i in range(N_CORES):
        shard = x[i * per_core : (i + 1) * per_core].reshape(S, H, W)
        in_maps.append({"x2": _pack_shard(shard), "afilt5": afilt})
    return in_maps


def kernel(x, kernel):
    from concourse.bass_utils import run_bass_kernel_spmd

    x = np.asarray(x, dtype=np.float32)
    kk = np.asarray(kernel, dtype=np.float32).reshape(-1)
    assert x.shape == (B, C, H, W)
    assert kk.shape == (TAPS,)

    key = kk.tobytes()
    if key not in _cache:
        _cache[key] = _build_with_taps(kk)
    nc = _cache[key]

    in_maps = make_in_maps(x, kk)
    res = run_bass_kernel_spmd(nc, in_maps, core_ids=list(range(N_CORES)))
    per_core = B // N_CORES
    out = np.empty((B, C, H, W), dtype=np.float32)
    for i in range(N_CORES):
        out[i * per_core : (i + 1) * per_core] = _unpack_out(
            res.results[i]["y2"]
        ).reshape(per_core, C, H, W)
    return out
